# revision 4
# baseline (speedup 1.0000x reference)
"""Distributed Trainium2 kernel for nn_AtlasMemoryLayer_33311766348071.

Sharding (hardcoded, per sharding_hint): data-parallel over batch B=2 and
over head-pairs (H=8 -> 4 pairs). 8 shards = 2 batches x 4 head-pairs, one
per NeuronCore. Each core runs the full chunked Atlas memory recurrence for
its (batch, 2-head) slice and produces a partial output projection
(T, C); the host sums the 4 per-batch partials (cheap: 8 * 512 * 512 adds).

Self-contained: shapes/constants hardcoded; no file reads.
"""

import os
import numpy as np
import jax

try:  # persistent compile cache so fresh processes reuse compiled NEFFs
    os.makedirs("/tmp/jax_cache", exist_ok=True)
    jax.config.update("jax_compilation_cache_dir", "/tmp/jax_cache")
    jax.config.update("jax_persistent_cache_min_compile_time_secs", 0.0)
except Exception:
    pass

import jax.numpy as jnp
import jax.lax as lax
from functools import partial

B, T, C = 2, 512, 512
H, D, E = 8, 64, 64
CS = 64
NCH = T // CS
WINDOW = 4
KERNEL = 4
NS_STEPS = 5
POLY_DEGREE = 2
MAX_LR = 0.1
GATE_BIAS = -2.0
HP = 2  # heads per core
NCORES = 8
_PE_COEFFS = [(8.156554524902461, -22.48329292557795, 15.878769915207462),
              (4.042929935166739, -2.808917465908714, 0.5000178451051316),
              (3.8916678022926607, -2.772484153217685, 0.5060648178503393),
              (3.285753657755655, -2.3681294933425376, 0.46449024233003106),
              (2.3465413258596377, -1.7097828382687081, 0.42323551169305323)]


def _gelu_derivative(x):
    x = x.astype(jnp.float32)
    cdf = 0.5 * (1.0 + lax.erf(x * 0.7071067811865476))
    pdf = jnp.exp(-0.5 * x * x) * 0.3989422804014327
    return cdf + x * pdf


def _linear_scan(h_init, gates, inputs):
    # h_init: (HP,...), gates: (CS,HP), inputs: (CS,HP,...); scan along axis 0
    g = gates
    for _ in range(inputs.ndim - gates.ndim):
        g = g[..., jnp.newaxis]
    first = g[0:1] * h_init[jnp.newaxis] + inputs[0:1]
    mod_inputs = jnp.concatenate([first, inputs[1:]], axis=0)
    mod_gates = jnp.concatenate([jnp.zeros_like(g[0:1]), g[1:]], axis=0)

    def combine(a, b):
        ga, xa = a
        gb, xb = b
        return (ga * gb, gb * xa + xb)

    _, h_all = lax.associative_scan(combine, (mod_gates, mod_inputs), axis=0)
    return h_all, h_all[-1]


def _omega_aggregate(u, gamma, window):
    t = u.shape[0]
    cum = jnp.cumsum(gamma * u, axis=0)
    if window >= t:
        return cum
    shifted = jnp.concatenate([jnp.zeros_like(cum[:window]), cum[:-window]], axis=0)
    return cum - shifted


def _polar_express(X, steps=NS_STEPS):
    dtype = X.dtype
    X = X.astype(jnp.float32)
    frob = jnp.sqrt(jnp.sum(X * X, axis=(-2, -1), keepdims=True) + 1e-12)
    X = X / (frob * 1.01 + 1e-06)
    # square (D == E): both reference branches are equivalent; use the else form
    for a, b, c in _PE_COEFFS[:steps]:
        A = X @ jnp.swapaxes(X, -2, -1)
        Bm = b * A + c * (A @ A)
        X = a * X + Bm @ X
    return X.astype(dtype)


def _rms_norm(x, eps=1e-06):
    dtype = x.dtype
    x = x.astype(jnp.float32)
    ms = jnp.mean(x * x, axis=-1, keepdims=True)
    return (x * lax.rsqrt(ms + eps)).astype(dtype)


def _short_conv(x, w, b):
    # x: (T, 2D) ; w: (2D, KERNEL) ; b: (2D,) -- causal depthwise conv along T
    xp = jnp.pad(x, ((KERNEL - 1, 0), (0, 0)))
    y = jnp.zeros_like(x)
    for j in range(KERNEL):
        y = y + xp[j:j + T, :] * w[:, j][jnp.newaxis, :]
    return y + b[jnp.newaxis, :]


def _poly_feat(x, coeffs):
    result = coeffs[0] * x
    xp = x
    for i in range(1, POLY_DEGREE):
        xp = xp * x
        result = result + coeffs[i] * xp
    return result


def _process_chunk(state, q_c, k_c, v_c, a_c, e_c, t_c, g_c):
    # state W1:(HP,D,E) W2:(HP,E,D); q/k/v_c:(CS,HP,D); gates a/e/t/g_c:(CS,HP)
    W1, W2, S_W1, S_W2 = state
    h = jnp.einsum('hed,chd->che', W2, k_c)
    act = jax.nn.gelu(h)
    y_pred = k_c + jnp.einsum('hde,che->chd', W1, act)
    err = y_pred - v_c
    scale = 2.0 / D
    err_f = err.astype(jnp.float32)
    u_W1 = (scale * jnp.einsum('chd,che->chde', err_f, act.astype(jnp.float32))).astype(err.dtype)
    chain = jnp.einsum('hde,chd->che', W1.astype(jnp.float32), err_f) * _gelu_derivative(h)
    u_W2 = (scale * jnp.einsum('che,chd->ched', chain, k_c.astype(jnp.float32))).astype(err.dtype)
    gw = g_c[..., jnp.newaxis, jnp.newaxis]
    u_W1 = _omega_aggregate(u_W1, gw, WINDOW)
    u_W2 = _omega_aggregate(u_W2, gw, WINDOW)
    mom_W1 = -(e_c[..., jnp.newaxis, jnp.newaxis] * u_W1)
    mom_W2 = -(e_c[..., jnp.newaxis, jnp.newaxis] * u_W2)
    theta = t_c
    all_S_W1, S_W1 = _linear_scan(S_W1, theta, mom_W1)
    all_S_W2, S_W2 = _linear_scan(S_W2, theta, mom_W2)
    all_S_W1o = _polar_express(all_S_W1, NS_STEPS)
    all_S_W2o = _polar_express(all_S_W2, NS_STEPS)
    alpha = a_c
    all_W1, W1 = _linear_scan(W1, alpha, all_S_W1o)
    all_W2, W2 = _linear_scan(W2, alpha, all_S_W2o)
    h_q = jnp.einsum('ched,chd->che', all_W2, q_c)
    y_c = q_c + jnp.einsum('chde,che->chd', all_W1, jax.nn.gelu(h_q))
    return y_c, (W1, W2, S_W1, S_W2)


def _core_fn(x_b, wq, wk, wv, cqw, cqb, ckw, ckb, cvw, cvb,
             gaw, gab, gew, geb, gtw, gtb, ggw, ggb,
             poly_coeffs, W1_init, W2_init, proj_slice):
    # x_b: (T, C); wq/wk/wv: (HP*D, C); c*w: (HP*D, KERNEL); c*b: (HP*D,)
    # g*w: (HP, C); g*b: (HP,); W1_init: (HP, D, E); W2_init: (HP, E, D)
    # proj_slice: (C, HP*D)
    q = _short_conv(x_b @ wq.T, cqw, cqb)
    k = _short_conv(x_b @ wk.T, ckw, ckb)
    v = _short_conv(x_b @ wv.T, cvw, cvb)
    q = _rms_norm(_poly_feat(q.reshape(T, HP, D), poly_coeffs))
    k = _rms_norm(_poly_feat(k.reshape(T, HP, D), poly_coeffs))
    v = v.reshape(T, HP, D)
    alpha = jax.nn.sigmoid(x_b @ gaw.T + gab + GATE_BIAS)  # (T, HP)
    eta = MAX_LR * jax.nn.sigmoid(x_b @ gew.T + geb + GATE_BIAS)
    theta = jax.nn.sigmoid(x_b @ gtw.T + gtb + GATE_BIAS)
    gamma = jax.nn.sigmoid(x_b @ ggw.T + ggb + GATE_BIAS)

    def chunk(t):
        return t.reshape(NCH, CS, *t.shape[1:])

    qs, ks_, vs = chunk(q), chunk(k), chunk(v)
    a_s, e_s, t_s, g_s = chunk(alpha), chunk(eta), chunk(theta), chunk(gamma)

    W1 = W1_init
    W2 = W2_init
    state0 = (W1, W2, jnp.zeros_like(W1), jnp.zeros_like(W2))

    def step(carry, xs):
        y_c, new_carry = _process_chunk(carry, *xs)
        return new_carry, y_c

    _, ys = lax.scan(step, state0, (qs, ks_, vs, a_s, e_s, t_s, g_s))
    y = ys.reshape(T, HP * D)
    return y @ proj_slice.T  # (T, C) partial contribution


_jitted = None


def _get_jitted():
    global _jitted
    if _jitted is None:
        _jitted = jax.jit(_core_fn)
    return _jitted


def kernel(x, c_q_w, c_k_w, c_v_w, c_proj_w, conv_q_w, conv_q_b, conv_k_w, conv_k_b,
           conv_v_w, conv_v_b, gate_alpha_w, gate_alpha_b, gate_eta_w, gate_eta_b,
           gate_theta_w, gate_theta_b, gate_gamma_w, gate_gamma_b, poly_coeffs,
           W1_init, W2_init):
    x = np.asarray(x, dtype=np.float32)
    c_q_w = np.asarray(c_q_w); c_k_w = np.asarray(c_k_w); c_v_w = np.asarray(c_v_w)
    c_proj_w = np.asarray(c_proj_w)
    conv_q_w = np.asarray(conv_q_w).reshape(H * D, KERNEL)
    conv_k_w = np.asarray(conv_k_w).reshape(H * D, KERNEL)
    conv_v_w = np.asarray(conv_v_w).reshape(H * D, KERNEL)
    conv_q_b = np.asarray(conv_q_b); conv_k_b = np.asarray(conv_k_b); conv_v_b = np.asarray(conv_v_b)
    gaw = np.asarray(gate_alpha_w); gab = np.asarray(gate_alpha_b)
    gew = np.asarray(gate_eta_w); geb = np.asarray(gate_eta_b)
    gtw = np.asarray(gate_theta_w); gtb = np.asarray(gate_theta_b)
    ggw = np.asarray(gate_gamma_w); ggb = np.asarray(gate_gamma_b)
    poly_coeffs = np.asarray(poly_coeffs)
    W1_init = np.asarray(W1_init); W2_init = np.asarray(W2_init)

    # Build per-core input stacks: core i -> batch i//4, heads [2*(i%4), 2*(i%4)+1]
    def stack(fn):
        return np.stack([fn(i) for i in range(NCORES)], axis=0)

    hsl = lambda i: slice((i % 4) * HP * D, ((i % 4) + 1) * HP * D)
    hslh = lambda i: slice((i % 4) * HP, ((i % 4) + 1) * HP)

    args = (
        stack(lambda i: x[i // 4]),
        stack(lambda i: c_q_w[hsl(i)]),
        stack(lambda i: c_k_w[hsl(i)]),
        stack(lambda i: c_v_w[hsl(i)]),
        stack(lambda i: conv_q_w[hsl(i)]),
        stack(lambda i: conv_q_b[hsl(i)]),
        stack(lambda i: conv_k_w[hsl(i)]),
        stack(lambda i: conv_k_b[hsl(i)]),
        stack(lambda i: conv_v_w[hsl(i)]),
        stack(lambda i: conv_v_b[hsl(i)]),
        stack(lambda i: gaw[hslh(i)]),
        stack(lambda i: gab[hslh(i)]),
        stack(lambda i: gew[hslh(i)]),
        stack(lambda i: geb[hslh(i)]),
        stack(lambda i: gtw[hslh(i)]),
        stack(lambda i: gtb[hslh(i)]),
        stack(lambda i: ggw[hslh(i)]),
        stack(lambda i: ggb[hslh(i)]),
        stack(lambda i: poly_coeffs),
        stack(lambda i: W1_init[hslh(i)]),
        stack(lambda i: W2_init[hslh(i)]),
        stack(lambda i: c_proj_w[:, hsl(i)]),
    )

    devs = jax.devices()[:NCORES]
    fn = _get_jitted()
    # Dispatch one shard per NeuronCore (async), then gather.
    futs = []
    for i in range(NCORES):
        core_args = [jax.device_put(a[i], devs[i]) for a in args]
        futs.append(fn(*core_args))
    partials = np.stack([np.asarray(f) for f in futs], axis=0)  # (8, T, C)
    out = np.empty((B, T, C), dtype=np.float32)
    for b in range(B):
        out[b] = partials[4 * b:4 * b + 4].sum(axis=0)
    return out
